# revision 19
# baseline (speedup 1.0000x reference)
"""Trainium2 Bass kernel for nn_DSNet (dense_cnn) — fp8 DoubleRow version.

Math: the reference computes
  ref  = conv1d(refer, w_seq, b_seq)            # (1, 512, 32768), k=3 over time
  seq  = concat([ref, x.T], time) -> (65536, 512)
  splits = seq.reshape(32768, 2, 512)
  s1   = relu(conv1d(splits, w1, b1))[:, 0, :]  # k=3 over the 512 axis
  h    = relu(s1 @ w2[:,:,1].T + b2)
  out  = sigmoid(h @ w3[:,:,1].T + b3)          # (32768, 64, 1)

Folding: for ref-half splits, linear_seq + concat + split + conv1 collapse into a
stride-2 4-tap conv on `refer` with host-precomputed Weff[d,i,tau] / beff[d]
(weight-only math). For the x half, conv1 along the 512 axis becomes banded
matmuls on 128-aligned xT windows plus tiny edge-fix matmuls.

Speed: all conv + mm2 matmuls run in fp8 e4m3 with MatmulPerfMode.DoubleRow
(2 contraction rows per PE cell -> 0.5 cycles/col). lhsT layout [K, 2, M],
rhs [K, 2, N]; psum stays fp32. Biases are folded into the conv matmuls via a
constant-1 rhs row so one relu can serve two d-blocks with different biases.
s1 is stored as fp8 [128, 2, 512] (two d-blocks per partition) so mm2 also
runs DoubleRow; mm3 runs bf16. Conv psums are merged pairwise into [128, 1024]
(2 banks) so each relu instruction covers two d-blocks; the two mm3 outputs
share one psum bank (ref rows 0:64, x rows 64:128) so one sigmoid covers both.

Elementwise (the bottleneck at ~12us/core) is split between ACT and DVE;
GPSIMD cannot read PSUM so it cannot help.

Sharding: splits sharded 8 ways; core c handles ref splits [2048c, 2048(c+1))
and x splits 16384 + [2048c, 2048(c+1)). Output per core [128, 2048] f32:
rows 0:64 ref outs, rows 64:128 x outs.
"""
import sys

import numpy as np

sys.path.insert(0, "/opt/trn_rl_repo")

D_IN, D_SEQ, D_H, D_OUT = 64, 512, 128, 64
T_REF = 32768
N_CORES = 8
NCH = 512  # splits per chunk
S1_FP8 = True  # fall back to bf16 s1 + bf16 mm2 if fp8 rounding hurts accuracy

_CACHE = {}


def _build_nc():
    import concourse.bacc as bacc
    import concourse.bass as bass
    import concourse.mybir as mybir
    import concourse.tile as tile

    f32 = mybir.dt.float32
    bf16 = mybir.dt.bfloat16
    e4 = mybir.dt.float8e4
    AF = mybir.ActivationFunctionType
    ALU = mybir.AluOpType
    DR = mybir.MatmulPerfMode.DoubleRow
    s1dt = e4 if S1_FP8 else bf16

    nc = bacc.Bacc("TRN2", target_bir_lowering=False, debug=False, num_devices=N_CORES)

    refer_d = nc.dram_tensor("refer_sl", [65, 4100], e4, kind="ExternalInput").ap()
    xwc_d = nc.dram_tensor("xwc", [4, 128, 4, 1024], e4, kind="ExternalInput").ap()
    edge_d = nc.dram_tensor("edge", [3, 4, 4, 1024], e4, kind="ExternalInput").ap()
    # ref conv lhsT, separate so the first convs' data arrives first:
    refw_d = nc.dram_tensor("refw", [65, 2048], e4, kind="ExternalInput").ap()
    # Other weights bundled (e4 bytes, other dtypes bitcast):
    #   cols 0:1024     wx: [128][i 2][slot 4][m 128] slot0=xmain,1=w2t0,2=w2t1,3=edge
    #   cols 1024:1152  w3 bf16 [128, 64]
    #   cols 1152:1160  wf f32 [128, 2] = b2 | b3dup
    wall_d = nc.dram_tensor("wall", [128, 1160], e4, kind="ExternalInput").ap()
    out_d = nc.dram_tensor("res", [128, 2048], bf16, kind="ExternalOutput").ap()

    with tile.TileContext(nc) as tc:
        with (
            tc.tile_pool(name="wp", bufs=1) as wp,
            tc.tile_pool(name="dp", bufs=2) as dp,
            tc.tile_pool(name="s1p", bufs=6) as s1p,
            tc.tile_pool(name="hp", bufs=2) as hp,
            tc.tile_pool(name="op", bufs=2) as op,
            tc.tile_pool(name="ppc", bufs=3, space=bass.MemorySpace.PSUM) as ppc,
            tc.tile_pool(name="pph", bufs=1, space=bass.MemorySpace.PSUM) as pph,
        ):
            # --- PE clock anchor (pe_busy_start latches at the first matmul;
            # the 3us ramp then runs concurrently with the DMA wait) + ACT
            # table warmup
            warm = wp.tile([1, 16], bf16)
            nc.gpsimd.memset(warm[:], 0.0)
            wact = wp.tile([1, 16], f32)
            nc.scalar.activation(wact[:], warm[0:1, 0:16], AF.Relu)
            nc.scalar.activation(wact[:], warm[0:1, 0:16], AF.Sigmoid)
            psw = ppc.tile([128, 1024], f32, tag="cv", name="psw")
            for _ in range(4):
                nc.tensor.matmul(
                    psw[0:1, 0:16], warm[0:1, 0:1], warm[0:1, :], start=True,
                    stop=True,
                )

            # --- weight + data loads (ordered for earliest first conv)
            refw_sb = wp.tile([65, 2048], e4)
            nc.sync.dma_start(refw_sb[:], refw_d)
            refer_sl = wp.tile([65, 4100], e4)
            nc.sync.dma_start(refer_sl[:, 0:2052], refer_d[:, 0:2052])
            wall = wp.tile([128, 1160], e4)
            nc.sync.dma_start(wall[:], wall_d)
            edge = wp.tile([3, 4, 4, 1024], e4)
            nc.sync.dma_start(edge[:], edge_d)

            xt_tiles = {}

            def load_xt(b):
                xt = dp.tile([128, 4, 1024], e4, tag="xt", name=f"xt_{b}")
                nc.sync.dma_start(xt[:], xwc_d[b])
                xt_tiles[b] = xt

            load_xt(0)
            nc.sync.dma_start(refer_sl[:, 2052:4100], refer_d[:, 2052:4100])
            load_xt(1)
            load_xt(2)
            load_xt(3)

            refw = refw_sb[:].rearrange("p (i s m) -> p i s m", i=2, s=8)
            wx = wall[:, 0:1024].rearrange("p (i s m) -> p i s m", i=2, s=4)
            w3 = wall[:, 1024:1152].bitcast(bf16)
            wf = wall[:, 1152:1160].bitcast(f32)
            b2v = wf[:, 0:1]
            b3v = wf[:, 1:2]

            def ref_conv(b, t):
                """Merged conv psum for ref d-blocks (2t, 2t+1) of chunk b."""
                ps = ppc.tile([128, 1024], f32, tag="cv", name=f"psr_{b}_{t}")
                for qh in (0, 1):
                    q = 2 * t + qh
                    reg = ps[:, 512 * qh:512 * qh + 512]
                    for g in (0, 1):
                        base = 1024 * b + 2 * g
                        np_ = 65 if g == 0 else 64
                        rhs = refer_sl[0:np_, base:base + 1024].rearrange(
                            "p (n i) -> p i n", i=2
                        )
                        nc.tensor.matmul(
                            reg, refw[0:np_, :, 2 * q + g, :], rhs,
                            start=(g == 0), stop=(g == 1), perf_mode=DR,
                        )
                return ps

            def x_conv(b, t):
                """Merged conv psum for x windows (2t, 2t+1) of chunk b."""
                xt = xt_tiles[b]
                ps = ppc.tile([128, 1024], f32, tag="cv", name=f"psx_{b}_{t}")
                for jh in (0, 1):
                    j = 2 * t + jh
                    reg = ps[:, 512 * jh:512 * jh + 512]
                    rhs = xt[:, j, :].rearrange("p (n i) -> p i n", i=2)
                    nc.tensor.matmul(
                        reg, wx[:, :, 0, :], rhs, start=True, stop=False, perf_mode=DR
                    )
                    erhs = edge[:, j, b, :].rearrange("p (n i) -> p i n", i=2)
                    nc.tensor.matmul(
                        reg, wx[0:3, :, 3, :], erhs, start=False, stop=True,
                        perf_mode=DR,
                    )
                return ps

            def s1_relu(ps, eng, name):
                s1 = s1p.tile([128, 2, 512], s1dt, tag="s1", name=name)
                flat = s1.rearrange("p a b -> p (a b)")
                if eng == 0:
                    nc.scalar.activation(flat, ps[:], AF.Relu)
                else:
                    nc.vector.tensor_scalar(flat, ps[:], 0.0, None, ALU.max)
                return s1

            def mm2(ph, half, s1_tiles):
                reg = ph[:, 512 * half:512 * half + 512]
                for t in (0, 1):
                    if S1_FP8:
                        nc.tensor.matmul(
                            reg, wx[:, :, 1 + t, :], s1_tiles[t][:],
                            start=(t == 0), stop=(t == 1), perf_mode=DR,
                        )
                    else:
                        for i in (0, 1):
                            nc.tensor.matmul(
                                reg, wx[:, i, 1 + t, :], s1_tiles[t][:, i, :],
                                start=(t == 0 and i == 0), stop=(t == 1 and i == 1),
                            )

            # Software-pipelined emission. Engine streams are in-order-ish, so
            # the late-dep tail of pair b-1 (mm3_x/sigmoid/dma) is emitted in
            # iteration b, and the ref-side tail (mm2 ref, h_ref relu, mm3
            # ref) runs early, overlapped with the x-side convs/relus. The
            # mm3 outputs reuse the h psum bank region cols 0:512 (WAR
            # ordered by Tile): mm3_ref rows 0:64 after h_ref reads it;
            # mm3_x rows 64:128.
            ph_tiles = {}

            def h_relu(ph, half, eng, name):
                hsb = hp.tile([128, 512], bf16, tag="hs", name=name)
                reg = ph[:, 512 * half:512 * half + 512]
                if eng == 0:
                    nc.scalar.activation(hsb[:], reg, AF.Relu, bias=b2v)
                else:
                    nc.vector.tensor_scalar(hsb[:], reg, b2v, 0.0, ALU.add, ALU.max)
                return hsb

            def tail_x(p):
                ph, hx = ph_tiles.pop(p)
                nc.tensor.matmul(ph[64:128, 0:512], w3[:, :], hx[:],
                                 start=True, stop=True)
                osb = op.tile([128, 512], bf16, tag="os", name=f"osb_{p}")
                nc.scalar.activation(osb[:], ph[:, 0:512], AF.Sigmoid, bias=b3v)
                nc.sync.dma_start(out_d[:, 512 * p:512 * p + 512], osb[:])

            for b in range(4):
                # ref convs + their relus (ACT gets t0, DVE t1)
                psr = [ref_conv(b, t) for t in (0, 1)]
                s1r = [
                    s1_relu(psr[0], 0, f"s1r_{b}_0"),
                    s1_relu(psr[1], 1, f"s1r_{b}_1"),
                ]
                # late tail of previous pair
                if b >= 1:
                    tail_x(b - 1)
                # x convs start while the ref-side tail runs
                psx = [x_conv(b, t) for t in (0, 1)]
                ph = pph.tile([128, 1024], f32, tag="h", name=f"ph_{b}")
                mm2(ph, 0, s1r)
                hr = h_relu(ph, 0, b % 2, f"hr_{b}")
                nc.tensor.matmul(ph[0:64, 0:512], w3[:, :], hr[:],
                                 start=True, stop=True)
                s1x = [
                    s1_relu(psx[0], 1, f"s1x_{b}_0"),
                    s1_relu(psx[1], 1 if b == 3 else 0, f"s1x_{b}_1"),
                ]
                mm2(ph, 1, s1x)
                hx = h_relu(ph, 1, (b + 1) % 2, f"hx_{b}")
                ph_tiles[b] = (ph, hx)
            tail_x(3)

    nc.compile()
    return nc


def _host_prep_weights(w_seq, b_seq, w1, b1, w2, b2, w3, b3):
    import ml_dtypes

    e4 = ml_dtypes.float8_e4m3
    bf = ml_dtypes.bfloat16

    w_seq64 = np.asarray(w_seq, np.float64)
    b_seq64 = np.asarray(b_seq, np.float64)
    w164 = np.asarray(w1, np.float64)
    b1f = float(np.asarray(b1).reshape(-1)[0])

    # Effective stride-2 4-tap conv weights for the ref half
    Weff = np.zeros((D_SEQ, D_IN, 4))
    beff = np.full(D_SEQ, b1f)
    for cc in (0, 1):
        for k in range(3):
            dlo, dhi = max(0, 1 - k), min(D_SEQ, D_SEQ + 1 - k)
            for kk in range(3):
                tau = cc + kk
                Weff[dlo:dhi, :, tau] += (
                    w164[0, cc, k] * w_seq64[dlo + k - 1:dhi + k - 1, :, kk]
                )
    for k in range(3):
        dlo, dhi = max(0, 1 - k), min(D_SEQ, D_SEQ + 1 - k)
        beff[dlo:dhi] += (w164[0, 0, k] + w164[0, 1, k]) * b_seq64[dlo + k - 1:dhi + k - 1]

    # ref conv lhsT [65, 2, 8, 128]: (q, g) slot 2q+g; group i -> tau 2g+i
    refw = np.zeros((65, 2, 8, 128), np.float64)
    for q in range(4):
        for g in (0, 1):
            for i in (0, 1):
                refw[0:64, i, 2 * q + g, :] = Weff[128 * q:128 * (q + 1), :, 2 * g + i].T
        refw[64, 0, 2 * q + 0, :] = beff[128 * q:128 * (q + 1)]

    # x conv lhsT main [128, 2, 128]: lhsT[m+k, c, m] = w1[c, k] for m+k<=127
    xmain = np.zeros((128, 2, 128), np.float64)
    for c in (0, 1):
        for k in range(3):
            for m in range(128):
                if m + k <= 127:
                    xmain[m + k, c, m] = w164[0, c, k]
    # x conv lhsT edge [3, 2, 128]: row0 = first edge input (d'=128j+127):
    #   m=126 tap k=2, m=127 tap k=1; row1 = second (d'=128j+128): m=127 k=2;
    #   row2 = ones row: b1 in group 0
    xedge = np.zeros((3, 2, 128), np.float64)
    for c in (0, 1):
        xedge[0, c, 126] = w164[0, c, 2]
        xedge[0, c, 127] = w164[0, c, 1]
        xedge[1, c, 127] = w164[0, c, 2]
    xedge[2, 0, :] = b1f

    # mm2 lhsT tiles [128, 2, 128] x2: w2t[p, i, t, m] = w2m[128(2t+i)+p, m]
    w2m = np.asarray(w2, np.float64)[:, :, 1].T  # (512, 128)
    wx = np.zeros((128, 2, 4, 128), np.float64)
    wx[:, :, 0, :] = xmain
    for t in (0, 1):
        for i in (0, 1):
            wx[:, i, 1 + t, :] = w2m[128 * (2 * t + i):128 * (2 * t + i + 1), :]
    wx[0:3, :, 3, :] = xedge

    w3m = np.asarray(w3, np.float64)[:, :, 1].T  # (128, 64)

    wf = np.zeros((128, 2), np.float64)
    wf[:, 0] = np.asarray(b2, np.float64)
    wf[0:64, 1] = np.asarray(b3, np.float64)
    wf[64:128, 1] = np.asarray(b3, np.float64)

    refw8 = np.ascontiguousarray(refw, e4).reshape(65, 2048)
    wall = np.zeros((128, 1160), e4)
    wb = wall.view(np.uint8)
    wb[:, 0:1024] = np.ascontiguousarray(wx, e4).reshape(128, 1024).view(np.uint8)
    wb[:, 1024:1152] = np.ascontiguousarray(w3m, bf).view(np.uint8)
    wb[:, 1152:1160] = np.ascontiguousarray(wf, np.float32).view(np.uint8)
    return refw8, wall


def _host_prep_core(c, refer8, x):
    import ml_dtypes

    e4 = ml_dtypes.float8_e4m3
    # refer_sl [65, 4100]: rows 0:64 refer cols [4096c-1, 4096c+4099), row 64 ones
    refer_sl = np.zeros((65, 4100), e4)
    lo, hi = 4096 * c - 1, 4096 * c + 4099
    glo, ghi = max(lo, 0), min(hi, T_REF)
    refer_sl[0:64, glo - lo:ghi - lo] = refer8[0, :, glo:ghi]
    refer_sl[64, :] = np.float64(1.0)

    # x windows: xTl[d, t] = x[4096c + t, d]
    xsl = x[0, 4096 * c:4096 * (c + 1), :]  # (4096, 512) f32
    xT8 = xsl.T.astype(e4)                  # (512, 4096)
    xwc = np.zeros((4, 128, 4, 1024), e4)
    for j in range(4):
        d0 = 128 * j - 1
        rlo = max(d0, 0)
        rhi = min(d0 + 128, D_SEQ)
        xwc[:, rlo - d0:rhi - d0, j, :] = (
            xT8[rlo:rhi, :].reshape(rhi - rlo, 4, 1024).transpose(1, 0, 2)
        )
    edge = np.zeros((3, 4, 4, 1024), e4)
    for j in range(4):
        edge[0, j, :, :] = xT8[128 * j + 127, :].reshape(4, 1024)
        if 128 * j + 128 < D_SEQ:
            edge[1, j, :, :] = xT8[128 * j + 128, :].reshape(4, 1024)
        edge[2, j, :, :] = np.float64(1.0)
    return refer_sl, xwc, edge


def kernel(refer, x, w_seq, b_seq, w1, b1, w2, b2, w3, b3):
    import ml_dtypes

    from concourse.bass_utils import run_bass_kernel_spmd

    refer = np.ascontiguousarray(np.asarray(refer), dtype=np.float32)
    x = np.ascontiguousarray(np.asarray(x), dtype=np.float32)
    refer8 = refer.astype(ml_dtypes.float8_e4m3)

    if "nc" not in _CACHE:
        _CACHE["nc"] = _build_nc()
    nc = _CACHE["nc"]

    refw8, wall = _host_prep_weights(w_seq, b_seq, w1, b1, w2, b2, w3, b3)
    in_maps = []
    for c in range(N_CORES):
        refer_sl, xwc, edge = _host_prep_core(c, refer8, x)
        in_maps.append(dict(
            refer_sl=refer_sl, xwc=xwc, edge=edge, refw=refw8, wall=wall
        ))

    res = run_bass_kernel_spmd(nc, in_maps, core_ids=list(range(N_CORES)))

    final = np.zeros((32768, D_OUT, 1), np.float32)
    for c in range(N_CORES):
        r = np.asarray(res.results[c]["res"], np.float32)  # (128, 2048) bf16
        final[2048 * c:2048 * (c + 1), :, 0] = r[0:64, :].T
        final[16384 + 2048 * c:16384 + 2048 * (c + 1), :, 0] = r[64:128, :].T
    return final
